# revision 30
# baseline (speedup 1.0000x reference)
"""Bass/Tile TRN2 kernel for nn_ExpressionAttentionLayer.

Math per batch b (B=8, G=2048, D=64):
    K_fused = concat([K_gene, K_expr], -1) @ WK_w.T + WK_b      # (G, D)
    Q_fused = concat([Q_gene, Q_expr], -1) @ WQ_w.T + WQ_b      # (G, D)
    A       = softmax(Q_fused @ K_fused.T / sqrt(D), axis=-1)
    out     = (A * M) @ V_expr                                   # (G, D)

Sharding: data-parallel over batch; core i handles batch i (B == n_cores == 8).
No collectives.

Per-core dataflow (v3):
  - All Q/K loads use the "(p s)" partition-contiguous DRAM layout (1KB+
    descriptors).  PE transpose-mode runs directly on fp32 (1 pass on
    cayman), so no pre-casts.
  - K side feeds both HWDGE rings first (kfT gates every logits matmul);
    its transposed chunks scatter to stride-16 column views to restore
    natural k order (k must line up with M's columns).
  - Q side keeps the chunk order: loop iteration qt computes the 128
    logical q rows {p*16 + qt}.  The M tile and the output tile for
    iteration qt use row-permuted DRAM views (M stays 8KB/partition
    descriptors), so no on-chip fixup is needed and iteration qt only
    depends on Q chunk qt -> the loop starts ~7us in.
  - M streams in as bf16 via SWDGE cast-DMA half-tiles on the gpsimd
    queue (its own ring; 16 MB of the ~19.4 MB total HBM read traffic).
  - Per q-tile, per 1024-col half h: logits psum(fp32) = Q_tile.T @
    K_fusedT (bf16); exp on ScalarE -> bf16 ex with fp32 row-sum accum
    (scale=1/sqrt(D) folded in; |logits| <~ 7 so no max-subtraction
    needed); em = ex * M_bf16 on VectorE (all-bf16 -> 2x DVE mode);
    PE-transpose em (bf16 psum); single DVE 2x copy to [k, q] sbuf
    tiles; accumulate out over k-tiles on PE; apply the softmax
    reciprocal on ScalarE while copying out of PSUM; DMA out on sync.
  - v4: the FINAL pair's AV matmuls are emitted inline at the end of the
    last loop iteration (not deferred past the loop), so the tail doesn't
    serialize AV -> normalize -> out-write after everything else retired.

Hardware clock note (measured): the PE clock is governed by HAM in fixed
~3.4us windows with a duty/credit budget — sustained 100% PE utilization
does NOT keep the 2.4 GHz grant (a 98%-busy window was observed being
demoted), and demotes carry a 12-16 window (~45us) half-clock penalty.
Average boost time per run is roughly fixed (~45us per ~110us kernel), so
total PE cycles — not gap elimination — is what matters; junk-matmul
"caulk" experiments (keeping PE busy through every window) made runs
SLOWER by spending the boost budget on junk work.
"""

from contextlib import ExitStack

import numpy as np

import concourse.bass as bass
import concourse.tile as tile
from concourse import bacc, mybir
from concourse.bass_utils import run_bass_kernel_spmd
from concourse.masks import make_identity

B, G, D = 8, 2048, 64
P = 128
NT = G // P  # 16 tiles of 128 rows
F32 = mybir.dt.float32
BF16 = mybir.dt.bfloat16
AF = mybir.ActivationFunctionType

N_CORES = 8


def _emit(ctx: ExitStack, tc: tile.TileContext, io: dict):
    nc = tc.nc

    singles = ctx.enter_context(tc.tile_pool(name="singles", bufs=1))
    ld = ctx.enter_context(tc.tile_pool(name="ld", bufs=4))

    # PSUM pools (8 banks total: ps_l 2x2 + ps_t 2x1 + ps_o 2x1 = 8)
    ps_l = ctx.enter_context(tc.tile_pool(name="ps_l", bufs=2, space="PSUM"))
    ps_t = ctx.enter_context(tc.tile_pool(name="ps_t", bufs=2, space="PSUM"))
    ps_o = ctx.enter_context(tc.tile_pool(name="ps_o", bufs=2, space="PSUM"))

    identity = singles.tile([P, P], F32)
    make_identity(nc, identity[:])
    identity_bf = singles.tile([P, P], BF16)
    nc.vector.tensor_copy(identity_bf[:], identity[:])

    # ---- M-tile SWDGE cast-DMA lookahead queue (bf16 in SBUF) ----
    # Row-permuted view: iteration qt covers logical q rows {p*16 + qt},
    # matching the Q-side chunk order (see module docstring).  8KB
    # contiguous per partition per half -> line-rate descriptors.
    mpool = ctx.enter_context(tc.tile_pool(name="mpool", bufs=7))
    m_r = io["M"].rearrange("(p s) k -> s p k", s=NT)
    mts = {}

    def issue_m(qt, poke=None):
        if qt < NT:
            mt = mpool.tile([P, G], BF16, tag="m", name="m")
            if poke is not None:
                # WAW dep: the DMA overwrites this poked region, so it cannot
                # start before the poke, which reads kfT block 0 — i.e. the
                # tail of the whole input pipeline (loads -> casts ->
                # transposes -> K proj).  On DVE: it issues no DMAs, so no
                # issue-order deadlock is possible, and the kfT dep pins it
                # late in the DVE queue behind the prologue copies.
                nc.vector.tensor_copy(mt[0:1, 0:D], poke[0:1, NT - 1, :])
            for h in range(2):
                nc.gpsimd.dma_start(
                    mt[:, h * 1024 : (h + 1) * 1024],
                    m_r[qt, :, h * 1024 : (h + 1) * 1024],
                )
            mts[qt] = mt

    # ---- weights as SWDGE bf16 cast-loads (first on the gpsimd queue —
    # tiny, and they keep both HWDGE rings free for the big K loads).
    # PE-transpose the two [64,64] halves in bf16, DVE copies them out.
    wk_nat = singles.tile([D, 2 * D], BF16, tag="wk_nat")
    wq_nat = singles.tile([D, 2 * D], BF16, tag="wq_nat")
    nc.gpsimd.dma_start(wk_nat[:], io["WK_w"][:, :])
    nc.gpsimd.dma_start(wq_nat[:], io["WQ_w"][:, :])
    wk_gTb = singles.tile([D, D], BF16, tag="wk_gTb")
    wk_eTb = singles.tile([D, D], BF16, tag="wk_eTb")
    wq_gTb = singles.tile([D, D], BF16, tag="wq_gTb")
    wq_eTb = singles.tile([D, D], BF16, tag="wq_eTb")
    for nat, dsts in ((wk_nat, (wk_gTb, wk_eTb)), (wq_nat, (wq_gTb, wq_eTb))):
        for h, dst in enumerate(dsts):
            psw = ps_t.tile([P, 8 * P], BF16, tag="ps_t", name="ps_w")[:D, :D]
            nc.tensor.transpose(
                psw[:], nat[:, h * D : (h + 1) * D], identity_bf[:D, :D]
            )
            nc.vector.tensor_copy(dst[:], psw[:])
    # biases duplicated onto both partition halves (the fused tensors live
    # on 128 partitions for logits row-packing)
    wkb = singles.tile([P, 1], F32, tag="wkb")
    wqb = singles.tile([P, 1], F32, tag="wqb")
    for half in range(2):
        nc.scalar.dma_start(wkb[half * D : (half + 1) * D], io["WK_b"][:, None])
        nc.scalar.dma_start(wqb[half * D : (half + 1) * D], io["WQ_b"][:, None])

    # ---- HAM warmup bridge: junk matmuls so the PE has no idle window
    # between kernel start and the first input transposes.  One PE-idle
    # HAM window (~3.4us) re-throttles the clock to 1.2 GHz and it has
    # been observed to STICK there for 50us+ — the bridge must reach the
    # data-dependent transposes.
    junk = singles.tile([P, 512], BF16, tag="junk")
    nc.vector.memset(junk[:], 0.0)
    for _ in range(10):
        psw = ps_o.tile([P, 512], F32, tag="ps_o", name="ps_warm")
        nc.tensor.matmul(psw[:], identity_bf[:], junk[:], start=True, stop=True)

    # ---- input loads.  Each HWDGE ring pays ~1.5-2us of fixed cost PER
    # DMA (serialized within the ring), so each ring carries exactly ONE
    # big transfer: K_gene on sync, K_expr on scalar, both in the "(p s)"
    # partition-contiguous layout (4KB descriptors).  The Q side and V go
    # through the SWDGE (gpsimd) queue as f32->bf16 cast-loads — SWDGE
    # descriptor-gen pipelines (~0.6us/DMA), the cast kills the separate
    # cast stage, and the queue's FIFO naturally holds the M stream (same
    # queue, behind them) out of the latency-critical input phase.
    bigs = {}
    bigs_bf = {}
    # K_gene fills the sync ring, K_expr the scalar ring (one big DMA
    # each — HWDGE rings serialize ~1.5-2us of fixed cost per DMA).
    for src_name, eng in (("K_gene", nc.sync), ("K_expr", nc.scalar)):
        big = ld.tile([P, NT, D], F32, tag=f"ld_{src_name}", name=f"ld_{src_name}")
        eng.dma_start(big[:], io[src_name].rearrange("(p s) d -> p s d", s=NT))
        bigs[src_name] = big

    # V as a bf16 SWDGE cast-load in "(p s)" layout (before Q: its
    # redistribute chain is longer), then per-chunk SBUF->SBUF DMAs on the
    # sync ring (no small-descriptor penalty on-chip) into the natural
    # [128, kt, 64] layout the AV matmuls need.
    v_ps = ld.tile([P, NT, D], BF16, tag="v_ps", name="v_ps")
    nc.gpsimd.dma_start(v_ps[:], io["V_expr"].rearrange("(p s) d -> p s d", s=NT))
    v_bf = singles.tile([P, NT, D], BF16, tag="v_bf")
    for kt in range(NT):
        nc.sync.dma_start(v_bf[:, kt, :], v_ps[8 * kt : 8 * kt + 8, :, :])

    # Q side as SWDGE bf16 cast-loads (kills the cast stage; the SWDGE
    # FIFO naturally holds the M stream behind them).
    for src_name in ("Q_gene", "Q_expr"):
        big_bf = ld.tile(
            [P, NT, D], BF16, tag=f"ldb_{src_name}", name=f"ldb_{src_name}"
        )
        nc.gpsimd.dma_start(
            big_bf[:], io[src_name].rearrange("(p s) d -> p s d", s=NT)
        )
        bigs_bf[src_name] = big_bf

    # M0..M4 queue up behind the weight/V/Q loads on the SWDGE FIFO; M5+
    # are issued from the loop body (gated by mpool buffer reuse).
    for _qt in range(5):
        issue_m(_qt)

    # K bf16 casts (the PE transposes are 1-pass bf16): gene on DVE,
    # expr on ACT, half-split so the first transposes start early.
    for src_name in ("K_gene", "K_expr"):
        bigs_bf[src_name] = ld.tile(
            [P, NT, D], BF16, tag=f"ldb_{src_name}", name=f"ldb_{src_name}"
        )
    for half in range(2):
        sl = (slice(None), slice(8 * half, 8 * half + 8), slice(None))
        nc.vector.tensor_copy(bigs_bf["K_gene"][sl], bigs["K_gene"][sl])
        nc.scalar.copy(bigs_bf["K_expr"][sl], bigs["K_expr"][sl])

    # ---- transpose K/Q gene+expr into bf16 [D, G] (d on partitions) ----
    # Chunk s of the "(p s)" load transposes to columns {p*16 + s}.  K side
    # scatters through a stride-16 view to restore natural k order; Q side
    # keeps chunk order (the q permutation is absorbed by the M/out views).
    kgT = singles.tile([D, G], BF16, tag="kgT")
    keT = singles.tile([D, G], BF16, tag="keT")
    qgT = singles.tile([D, G], BF16, tag="qgT")
    qeT = singles.tile([D, G], BF16, tag="qeT")
    # fused tensors duplicated on BOTH partition halves (rows 64-127 =
    # copy of rows 0-63) so the loop can row-pack two 64-contraction
    # logits matmuls into the PE array concurrently (tile_position).
    kfT = singles.tile([P, G], BF16, tag="kfT")
    qfT = singles.tile([P, G], BF16, tag="qfT")

    def emit_transposes(side, gT, eT, j):
        # Both sides store CHUNK order: chunk s -> columns [s*128,(s+1)*128),
        # holding g = p*16 + s at position p.  All copies are contiguous.
        # K copies all on DVE (ACT's early queue is busy with Q casts and
        # the M pokes); Q copies split DVE/ACT.
        if side == "K":
            engines = ((0, gT, nc.vector), (1, eT, nc.vector))
        else:
            engines = ((0, gT, nc.vector), (1, eT, nc.scalar))
        for c, dstT, ceng in engines:
            big = bigs_bf[f"{side}_gene" if c == 0 else f"{side}_expr"]
            ps = ps_t.tile([P, 8 * P], BF16, tag="ps_t", name="ps_tr")[:D, : 4 * P]
            for i in range(4):
                s = 4 * j + i
                nc.tensor.transpose(
                    ps[:, i * P : (i + 1) * P], big[:, s, :], identity_bf[:]
                )
            dst = dstT[:, j * 512 : (j + 1) * 512].rearrange("d (i p) -> d i p", i=4)
            src = ps[:].rearrange("d (i p) -> d i p", i=4)
            if ceng is nc.vector:
                ceng.tensor_copy(dst, src)
            else:
                ceng.copy(dst, src)

    def emit_proj(gT, eT, wgT, weT, b_sb, fT, j, permute):
        # K side: the chunk->natural column reorder rides the projection's
        # moving-operand AP (the PE AP walker streams any column order at
        # line rate): natural column g <-> chunk position (g%16)*128 + g//16.
        if permute:
            rg = gT[:].rearrange("d (s p) -> d p s", s=NT)[:, j * 32 : (j + 1) * 32, :]
            re = eT[:].rearrange("d (s p) -> d p s", s=NT)[:, j * 32 : (j + 1) * 32, :]
        else:
            rg = gT[:, j * 512 : (j + 1) * 512]
            re = eT[:, j * 512 : (j + 1) * 512]
        # project into BOTH psum partition halves (col tile positions 0 and
        # 64 run concurrently) so one per-partition-aligned copy fills both
        # halves of the duplicated fused tensor.
        psj = ps_o.tile([P, 512], F32, tag="ps_o", name="ps_pj")
        for half in range(2):
            hsl = slice(half * D, (half + 1) * D)
            nc.tensor.matmul(psj[hsl, :], wgT[:], rg, start=True, stop=False)
            nc.tensor.matmul(psj[hsl, :], weT[:], re, start=False, stop=True)
        if permute:
            # bias-add + bf16 cast on DVE (ACT is busier at this point)
            nc.vector.tensor_scalar_add(
                fT[:, j * 512 : (j + 1) * 512], psj[:], b_sb[:, 0:1]
            )
        else:
            nc.scalar.activation(
                fT[:, j * 512 : (j + 1) * 512], psj[:], AF.Identity, bias=b_sb[:, 0:1]
            )

    for j in range(4):
        emit_transposes("K", kgT, keT, j)
    for j in range(4):
        # K projections need all K chunks (each natural 512-block reads
        # one position from every chunk); they gate every logits matmul,
        # so they come before the Q-side PE work.
        emit_proj(kgT, keT, wk_gTb, wk_eTb, wkb, kfT, j, permute=True)
    # Q block 0 only — blocks 1-3 are emitted inside the first loop
    # iterations (the loop is DMA-bound, PE has slack), so the loop starts
    # as soon as block 0 is projected.  Q keeps chunk order end-to-end
    # (M/out row views absorb it): block j only needs chunks 4j..4j+3.
    emit_transposes("Q", qgT, qeT, 0)
    emit_proj(qgT, qeT, wq_gTb, wq_eTb, wqb, qfT, 0, permute=False)

    def emit_q_block(j):
        emit_transposes("Q", qgT, qeT, j)
        emit_proj(qgT, qeT, wq_gTb, wq_eTb, wqb, qfT, j, permute=False)

    # ---- main attention loop (fully per-q-tile pipelined) ----
    epool = ctx.enter_context(tc.tile_pool(name="epool", bufs=4))
    empool = ctx.enter_context(tc.tile_pool(name="empool", bufs=4))
    tpool = ctx.enter_context(tc.tile_pool(name="tpool", bufs=4))
    opool = ctx.enter_context(tc.tile_pool(name="opool", bufs=4))
    rspool = ctx.enter_context(tc.tile_pool(name="rspool", bufs=4))

    out_r = io["out"].rearrange("(p s) d -> s p d", s=NT)
    scale = 1.0 / np.sqrt(np.float32(D))

    # Pair-loop: tiles A=2t, B=2t+1 are processed together; their logits
    # matmuls row-pack into the PE array (A in rows 0-63, B in rows
    # 64-127, concurrent) — halves the logits' PE time, which matters
    # because the PE clock can stick at 1.2 GHz (HAM erratum) and the
    # per-tile PE work must still fit under the ~2.9us M-stream cadence.
    # The previous pair's AV matmuls are emitted between this pair's h0
    # and its transposes so the PE never stalls at a group barrier.
    pending = []  # [(qt, emt, recip), ...]

    def emit_av(pend):
        qt_p, emt_p, recip_p = pend
        # out[q, d] += expM^T_chunk.T @ V  (lhsT=emt chunk: 128 bf16 cols -> FWL)
        pso = ps_o.tile([P, 512], F32, tag="ps_o", name="ps_av")[:, :D]
        for kt in range(NT):
            nc.tensor.matmul(
                pso[:],
                emt_p[:, kt, :],
                v_bf[:, kt, :],
                start=(kt == 0),
                stop=(kt == NT - 1),
            )
        ob = opool.tile([P, D], F32, tag="ob")
        # apply softmax denominator while copying out of PSUM
        nc.scalar.activation(ob[:], pso[:], AF.Copy, bias=0.0, scale=recip_p[:, 0:1])
        # out-writes ride the scalar ring (the sync ring carries the V
        # redistribute chain in the prologue)
        nc.scalar.dma_start(out_r[qt_p], ob[:])

    for t in range(NT // 2):
        qts = (2 * t, 2 * t + 1)
        mtab = [mts.pop(qt) for qt in qts]
        issue_m(2 * t + 5)
        issue_m(2 * t + 6)

        exab = [epool.tile([P, G], BF16, tag="ex", name="ex") for _ in range(2)]
        emab = [empool.tile([P, G], BF16, tag="em", name="em") for _ in range(2)]
        emtab = [tpool.tile([P, NT, P], BF16, tag="emt", name="emt") for _ in range(2)]
        rsab = [
            [rspool.tile([P, 1], F32, tag=f"rs{a}{h}", name=f"rs{a}{h}") for h in range(2)]
            for a in range(2)
        ]

        for h in range(2):
            hsl = slice(h * 1024, (h + 1) * 1024)
            psls = []
            for a in range(2):
                # row-packed logits: lhsT/rhs from partition half a
                psl = ps_l.tile([P, 1024], F32, tag="ps_l")
                asl = slice(a * D, (a + 1) * D)
                for n in range(2):
                    nc.tensor.matmul(
                        psl[:, n * 512 : (n + 1) * 512],
                        qfT[asl, qts[a] * P : (qts[a] + 1) * P],
                        kfT[asl, (2 * h + n) * 512 : (2 * h + n + 1) * 512],
                        start=True,
                        stop=True,
                    )
                psls.append(psl)
            for a in range(2):
                # exp -> bf16 with fp32 row-sum accumulation
                nc.scalar.activation(
                    exab[a][:, hsl],
                    psls[a][:],
                    AF.Exp,
                    scale=float(scale),
                    accum_out=rsab[a][h][:],
                )
                # bf16 x bf16 -> bf16 multiply: DVE 2x mode
                nc.vector.tensor_mul(
                    emab[a][:, hsl], exab[a][:, hsl], mtab[a][:, hsl]
                )

            # previous pair's AV runs on PE between this pair's halves
            if h == 0:
                for pend in pending:
                    emit_av(pend)
                pending = []

            for a in range(2):
                # transpose this half's 8 [128,128] blocks; 2x DVE copy out
                pst = ps_t.tile([P, 8 * P], BF16, tag="ps_t")
                for k in range(8):
                    kt = 8 * h + k
                    nc.tensor.transpose(
                        pst[:, k * P : (k + 1) * P],
                        emab[a][:, kt * P : (kt + 1) * P],
                        identity_bf[:],
                    )
                nc.vector.tensor_copy(
                    emtab[a][:, 8 * h : 8 * h + 8, :],
                    pst[:].rearrange("p (a b) -> p a b", a=8),
                )

        for a in range(2):
            rsum = rspool.tile([P, 1], F32, tag=f"rsum{a}", name="rsum")
            nc.vector.tensor_add(rsum[:], rsab[a][0][:], rsab[a][1][:])
            recip = rspool.tile([P, 1], F32, tag=f"recip{a}", name="recip")
            nc.vector.reciprocal(recip[:], rsum[:])
            pending.append((qts[a], emtab[a], recip))

        # deferred Q-side blocks ride the loop's PE slack (loop is DMA-bound)
        if t < 3:
            emit_q_block(t + 1)

        # final pair: AV inline so the tail doesn't serialize behind the loop
        if t == NT // 2 - 1:
            for pend in pending:
                emit_av(pend)
            pending = []

    for pend in pending:
        emit_av(pend)


def _build():
    # Bacc (not plain Bass): its compile() legalizes sync waits
    # (move_matmul_waits_to_ldweights + generate_event_semaphores) which
    # walrus codegen requires (max 1 wait per instruction).
    nc = bacc.Bacc("TRN2", target_bir_lowering=False, debug=False)
    io = {}
    for name in ("Q_gene", "K_gene", "Q_expr", "K_expr", "V_expr"):
        io[name] = nc.dram_tensor(name, [G, D], F32, kind="ExternalInput").ap()
    io["M"] = nc.dram_tensor("M", [G, G], F32, kind="ExternalInput").ap()
    for name in ("WK_w", "WQ_w"):
        io[name] = nc.dram_tensor(name, [D, 2 * D], F32, kind="ExternalInput").ap()
    for name in ("WK_b", "WQ_b"):
        io[name] = nc.dram_tensor(name, [D], F32, kind="ExternalInput").ap()
    io["out"] = nc.dram_tensor("out", [G, D], F32, kind="ExternalOutput").ap()

    with tile.TileContext(nc) as tc:
        with ExitStack() as ctx:
            _emit(ctx, tc, io)
    nc.compile()
    return nc


_NC = None


def _get_nc():
    global _NC
    if _NC is None:
        _NC = _build()
    return _NC


def kernel(**inputs) -> np.ndarray:
    return run_kernel_with_results(**inputs)[0]


def run_kernel_with_results(trace=False, **inputs):
    """Returns (full_output, BassKernelResults)."""
    nc = _get_nc()
    per_core_names = ("Q_gene", "K_gene", "Q_expr", "K_expr", "V_expr", "M")
    shared_names = ("WK_w", "WK_b", "WQ_w", "WQ_b")
    arrs = {k: np.ascontiguousarray(np.asarray(v), dtype=np.float32) for k, v in inputs.items()}
    in_maps = []
    for c in range(N_CORES):
        im = {n: arrs[n][c] for n in per_core_names}
        for n in shared_names:
            im[n] = arrs[n]
        in_maps.append(im)
    res = run_bass_kernel_spmd(nc, in_maps, list(range(N_CORES)), trace=trace)
    out = np.stack([res.results[c]["out"] for c in range(N_CORES)], axis=0)
    return out.astype(np.float32), res

